# revision 1
# baseline (speedup 1.0000x reference)
"""Trainium2 Bass kernel for nn_DeconvDft2dLayer.

y = irfft2(gmf * rfft2(pad(x)))  with x (64,512,512), w (3,3), y (64,768,768).

Strategy: data-parallel over batch (8 samples per NeuronCore). Per sample the
FFTs are evaluated as DFT matmuls on the tensor engine (fp32r, full rate):

  A : S1^T[w,k] = sum_h x[h,w] W2[h,k]            k in [0,385)   (fft-H, halved
      via Hermitian symmetry of the real input)
  B1: S[k,j]    = sum_w S1[k,w] C1[w,j]           k in [0,384)
  B2: S'[k',j]  = sum_w S1[k',w] conj(C1)[w,j]    k' in [1,385)
  C : T[r,j] (768 rows) = gmf[rho(r)] * X,  X = S rows (r<384) or conj(S') rows
      (rho(r) = r for r<384, 1151-r otherwise) -- elementwise on VectorE
  D : U^T[j,n]  = sum_r T[r,j] Atil[r,n],  Atil[r,n] = e^{2i pi n rho(r)/768}/768^2
  E : y[n,m]    = sum_j Ure[j,n] Bre[j,m] + Uim[j,n] Bimn[j,m]
      Bre = w_j cos(2 pi j m/768), Bimn = -w_j sin(2 pi j m/768)

gmf and the DFT matrices are tiny 3x3-derived constants computed host-side
(float64) and replicated to all cores; no cross-device communication.
All DRAM tensors are host-packed in the exact SBUF tile layout so every DMA is
128 large contiguous descriptors (the sync engine's descriptor generation is
the limiter otherwise).
"""
import os

import numpy as np

import concourse.bacc as bacc
import concourse.mybir as mybir
import concourse.tile as tile
from concourse.bass_utils import run_bass_kernel_spmd

F32 = mybir.dt.float32
F32R = mybir.dt.float32r

HP = 768          # padded grid
J = 385           # rfft half length (768//2+1)
JP = 386          # padded to even for fp32r free-dim constraint
NS = 8            # samples per core
NCORES = 8

LAST_EXEC_NS = None
LAST_RESULTS = None


def _build_constants(w):
    """Host-side constants (float64 -> float32), packed in SBUF tile layout."""
    w = np.asarray(w, np.float64)
    hm1 = np.zeros((HP, HP)); hm1[:3, :3] = w
    gm1f = 1.0 / np.fft.rfft2(hm1)
    gm2f = np.roll(gm1f[::-1, :], shift=1, axis=0)
    gm3f = np.roll(gm1f[:, ::-1], shift=1, axis=1)
    gm4f = np.roll(gm3f[::-1, :], shift=1, axis=0)
    gmf = (gm1f * gm2f) * (gm3f * gm4f)          # (768, 385) complex

    h = np.arange(512)
    k = np.arange(J)
    ph = np.exp(-2j * np.pi * (np.outer(h + 128, k) % HP) / HP)   # (512,385)
    j = np.arange(J)
    c1 = np.zeros((3, 512, JP))
    c1[0, :, :J] = ph.real            # C1 == W2 (same 512x385 phase table)
    c1[1, :, :J] = ph.imag
    c1[2, :, :J] = -ph.imag

    r = np.arange(HP)
    rho = np.where(r < 384, r, 1151 - r)
    pq = np.zeros((2, HP, JP))
    pq[0, :, :J] = gmf.real[rho, :]
    pq[1, :, :J] = gmf.imag[rho, :]

    n = np.arange(HP)
    pa = np.exp(2j * np.pi * (np.outer(rho, n) % HP) / HP) / (HP * HP)
    atil = np.stack([pa.real, pa.imag, pa.real + pa.imag])   # (3, 768, 768)

    m = np.arange(HP)
    wj = np.where((j == 0) | (j == 384), 1.0, 2.0)
    ang = 2 * np.pi * (np.outer(j, m) % HP) / HP
    bre = wj[:, None] * np.cos(ang)              # (385, 768)
    bimn = -wj[:, None] * np.sin(ang)
    bmat = np.stack([bre[:384], bimn[:384]])     # (2, 384, 768)
    b384 = np.stack([bre[384], bimn[384]])       # (2, 768): p0=Bre, p1=Bimn

    f = np.float32
    return {
        # packed to SBUF layouts: leading dim = partition
        "c1": np.ascontiguousarray(c1.reshape(3, 4, 128, JP).transpose(2, 0, 1, 3), f),
        "pq": np.ascontiguousarray(pq.reshape(2, 6, 128, JP).transpose(2, 0, 1, 3), f),
        "atil": np.ascontiguousarray(atil.reshape(3, 6, 128, HP).transpose(2, 0, 1, 3), f),
        "bmat": np.ascontiguousarray(bmat.reshape(2, 3, 128, HP).transpose(2, 0, 1, 3), f),
        "b384": np.ascontiguousarray(b384, f),
    }


def _build_program(ns=NS):
    nc = bacc.Bacc("TRN2", target_bir_lowering=False, debug=False,
                   num_devices=NCORES)
    x_ext = nc.declare_dram_parameter("x", [ns, 128, 4, 512], F32R, isOutput=False)
    y_ext = nc.declare_dram_parameter("y", [ns, 128, 6, HP], F32, isOutput=True)
    c1_ext = nc.declare_dram_parameter("c1", [128, 3, 4, JP], F32R, isOutput=False)
    pq_ext = nc.declare_dram_parameter("pq", [128, 2, 6, JP], F32, isOutput=False)
    atil_ext = nc.declare_dram_parameter("atil", [128, 3, 6, HP], F32R, isOutput=False)
    bmat_ext = nc.declare_dram_parameter("bmat", [128, 2, 3, HP], F32R, isOutput=False)
    b384_ext = nc.declare_dram_parameter("b384", [2, HP], F32R, isOutput=False)

    MUL = mybir.AluOpType.mult
    ADD = mybir.AluOpType.add
    SUB = mybir.AluOpType.subtract

    # tmat component order: 0 = -Tim, 1 = Tre, 2 = Tim, 3 = Tre+Tim  (the two
    # Nyquist lhsT pairs (Tre,Tim)=[1:3] and (-Tim,Tre)=[0:2] are contiguous;
    # Tsum feeds the Karatsuba M3 = Tsum @ (Are+Aim))
    TIMN, TRE, TIM, TSUM = 0, 1, 2, 3

    with tile.TileContext(nc) as tc:
        with tc.tile_pool(name="const", bufs=1) as cpool, \
             tc.tile_pool(name="data", bufs=1) as dpool, \
             tc.tile_pool(name="udata", bufs=2) as upool, \
             tc.tile_pool(name="xin", bufs=1) as xpool, \
             tc.tile_pool(name="yout", bufs=2) as ypool, \
             tc.tile_pool(name="scr", bufs=2) as spool, \
             tc.tile_pool(name="psum", bufs=8, space="PSUM") as ppool:

            # sample-0 input first so stage A can start during const loads
            xts = []
            xt0 = xpool.tile([128, 4, 512], F32R, tag="x")
            nc.sync.dma_start(out=xt0[:], in_=x_ext[0])
            xts.append(xt0)

            c1_t = cpool.tile([128, 3, 4, JP], F32R, tag="c1")
            nc.sync.dma_start(out=c1_t[:], in_=c1_ext[:])
            pq_t = cpool.tile([128, 2, 6, JP], F32, tag="pq")
            nc.sync.dma_start(out=pq_t[:], in_=pq_ext[:])
            a_t = cpool.tile([128, 3, 6, HP], F32R, tag="atil")
            nc.sync.dma_start(out=a_t[:], in_=atil_ext[:])
            b_t = cpool.tile([128, 2, 3, HP], F32R, tag="bmat")
            nc.sync.dma_start(out=b_t[:], in_=bmat_ext[:])
            b384_t = cpool.tile([2, HP], F32R, tag="b384")
            nc.sync.dma_start(out=b384_t[:], in_=b384_ext[:])

            def mm(ps, lhsT, rhs, start, stop):
                nc.tensor.matmul(ps, lhsT=lhsT, rhs=rhs, start=start, stop=stop)

            for b in range(ns):
                xt = xts[b]
                if b + 1 < ns:   # prefetch next sample
                    nxt = xpool.tile([128, 4, 512], F32R, tag="x")
                    nc.sync.dma_start(out=nxt[:], in_=x_ext[b + 1])
                    xts.append(nxt)

                s1 = dpool.tile([128, 2, 4, JP], F32R, tag="s1")
                tmat = dpool.tile([128, 4, 6, JP], F32R, tag="tmat")
                ut = dpool.tile([128, 2, 3, HP], F32R, tag="ut")
                u384 = dpool.tile([2, HP], F32R, tag="u384")

                # ---- stage A ----
                for comp in range(2):
                    for wc in range(4):
                        ps = ppool.tile([128, JP], F32, tag="ps")
                        for hc in range(4):
                            mm(ps[:], xt[:, hc, wc * 128:(wc + 1) * 128],
                               c1_t[:, comp, hc, :], hc == 0, hc == 3)
                        nc.vector.tensor_copy(out=s1[:, comp, wc, :], in_=ps[:])

                # ---- stages B + C, 6 chunks of T ----
                for c in range(6):
                    mirror = c >= 3
                    lo = (c - 3) * 128 + 1 if mirror else c * 128
                    ksl = slice(lo, lo + 128)
                    ps_re = ppool.tile([128, JP], F32, tag="ps")
                    ps_im = ppool.tile([128, JP], F32, tag="ps")
                    # real part of S (or S')
                    for wc in range(4):
                        mm(ps_re[:], s1[:, 0, wc, ksl], c1_t[:, 0, wc, :], wc == 0, False)
                    for wc in range(4):
                        # S: + S1im @ (-C1im) ; S': + S1im @ (+C1im)
                        mm(ps_re[:], s1[:, 1, wc, ksl],
                           c1_t[:, 1 if mirror else 2, wc, :], False, wc == 3)
                    # imag part
                    for wc in range(4):
                        # S: + S1re @ C1im ; S': + S1re @ (-C1im)
                        mm(ps_im[:], s1[:, 0, wc, ksl],
                           c1_t[:, 2 if mirror else 1, wc, :], wc == 0, False)
                    for wc in range(4):
                        mm(ps_im[:], s1[:, 1, wc, ksl], c1_t[:, 0, wc, :], False, wc == 3)

                    # ---- stage C on this chunk ----
                    t_re = tmat[:, TRE, c, :]
                    t_im = tmat[:, TIM, c, :]
                    t_imn = tmat[:, TIMN, c, :]
                    scr = spool.tile([128, JP], F32, tag="scr")
                    scr2 = spool.tile([128, JP], F32, tag="scr")
                    nc.vector.tensor_tensor(out=t_re, in0=pq_t[:, 0, c, :], in1=ps_re[:], op=MUL)
                    nc.vector.tensor_tensor(out=scr[:], in0=pq_t[:, 1, c, :], in1=ps_im[:], op=MUL)
                    nc.vector.tensor_tensor(out=t_re, in0=t_re, in1=scr[:],
                                            op=ADD if mirror else SUB)
                    nc.vector.tensor_tensor(out=t_im, in0=pq_t[:, 1, c, :], in1=ps_re[:], op=MUL)
                    nc.vector.tensor_tensor(out=scr2[:], in0=pq_t[:, 0, c, :], in1=ps_im[:], op=MUL)
                    nc.vector.tensor_tensor(out=t_im, in0=t_im, in1=scr2[:],
                                            op=SUB if mirror else ADD)
                    nc.vector.tensor_scalar_mul(t_imn, t_im, -1.0)
                    nc.vector.tensor_tensor(out=tmat[:, TSUM, c, :], in0=t_re,
                                            in1=t_im, op=ADD)

                # ---- stage D, full j-chunks (Karatsuba: 18 matmuls/group) ----
                for jc in range(3):
                    jsl = slice(jc * 128, jc * 128 + 128)
                    for nh in range(2):
                        nsl = slice(nh * 384, nh * 384 + 384)
                        pm1 = ppool.tile([128, 384], F32, tag="ps")
                        pm2 = ppool.tile([128, 384], F32, tag="ps")
                        pm3 = ppool.tile([128, 384], F32, tag="ps")
                        for rc in range(6):
                            mm(pm1[:], tmat[:, TRE, rc, jsl], a_t[:, 0, rc, nsl], rc == 0, rc == 5)
                        for rc in range(6):
                            mm(pm2[:], tmat[:, TIM, rc, jsl], a_t[:, 1, rc, nsl], rc == 0, rc == 5)
                        for rc in range(6):
                            mm(pm3[:], tmat[:, TSUM, rc, jsl], a_t[:, 2, rc, nsl], rc == 0, rc == 5)
                        # Ure = M1 - M2 ; Uim = M3 - M1 - M2
                        scrd = spool.tile([128, JP], F32, tag="scr")
                        nc.scalar.copy(scrd[:, :384], pm2[:, :])
                        nc.vector.tensor_tensor(out=ut[:, 0, jc, nsl], in0=pm1[:],
                                                in1=scrd[:, :384], op=SUB)
                        nc.vector.tensor_tensor(out=ut[:, 1, jc, nsl], in0=pm3[:],
                                                in1=ut[:, 0, jc, nsl], op=SUB)
                        nc.vector.scalar_tensor_tensor(
                            out=ut[:, 1, jc, nsl], in0=scrd[:, :384], scalar=-2.0,
                            in1=ut[:, 1, jc, nsl], op0=MUL, op1=ADD)

                # ---- stage D, Nyquist column j=384: both comps in one psum ----
                for nh in range(2):
                    nsl = slice(nh * 384, nh * 384 + 384)
                    ps_u = ppool.tile([2, 384], F32, tag="ps")
                    for rc in range(6):
                        # rows += (Tre, Tim) @ Are
                        mm(ps_u[:], tmat[:, TRE:TIM + 1, rc, 384], a_t[:, 0, rc, nsl],
                           rc == 0, False)
                    for rc in range(6):
                        # rows += (-Tim, Tre) @ Aim  ->  (Ure, Uim)
                        mm(ps_u[:], tmat[:, TIMN:TIM, rc, 384], a_t[:, 1, rc, nsl],
                           False, rc == 5)
                    nc.scalar.copy(u384[:, nsl], ps_u[:, :])

                # ---- stage E ----
                for nch in range(6):
                    nsl = slice(nch * 128, nch * 128 + 128)
                    ytc = ypool.tile([128, HP], F32, tag="y")
                    for mh in range(2):
                        msl = slice(mh * 384, mh * 384 + 384)
                        ps_y = ppool.tile([128, 384], F32, tag="ps")
                        for jc in range(3):
                            mm(ps_y[:], ut[:, 0, jc, nsl], b_t[:, 0, jc, msl], jc == 0, False)
                        for jc in range(3):
                            mm(ps_y[:], ut[:, 1, jc, nsl], b_t[:, 1, jc, msl], False, False)
                        mm(ps_y[:], u384[:, nsl], b384_t[:, msl], False, True)
                        nc.vector.tensor_copy(out=ytc[:, msl], in_=ps_y[:])
                    nc.sync.dma_start(out=y_ext[b, :, nch, :], in_=ytc[:])

    nc.compile()
    return nc


_PROGRAM_CACHE = {}


def kernel(x, w, trace=False):
    global LAST_EXEC_NS, LAST_RESULTS
    x = np.asarray(x, np.float32)
    B = x.shape[0]
    # pack to the SBUF tile layout: x_dev[b, p, c, w] = x[b, c*128+p, w]
    x_dev = np.ascontiguousarray(
        x.reshape(B, 4, 128, 512).transpose(0, 2, 1, 3))
    consts = _build_constants(w)
    if NS not in _PROGRAM_CACHE:
        _PROGRAM_CACHE[NS] = _build_program(NS)
    nc = _PROGRAM_CACHE[NS]
    in_maps = []
    for core in range(NCORES):
        m = {"x": x_dev[core * NS:(core + 1) * NS]}
        m.update(consts)
        in_maps.append(m)
    if trace:
        os.environ.pop("BASS_NEVER_TRACE", None)
        res = run_bass_kernel_spmd(nc, in_maps, list(range(NCORES)), trace=True)
    else:
        # profiling needs the antenv NTFF shim; never let a stray BASS_TRACE
        # env var route us down that path during plain runs
        os.environ["BASS_NEVER_TRACE"] = "1"
        try:
            res = run_bass_kernel_spmd(nc, in_maps, list(range(NCORES)), trace=False)
        finally:
            os.environ.pop("BASS_NEVER_TRACE", None)
    LAST_EXEC_NS = res.exec_time_ns
    LAST_RESULTS = res
    # unshard: y_dev[b, p, c, m] -> y[b, c*128+p, m]
    y_dev = np.concatenate([res.results[i]["y"] for i in range(NCORES)], axis=0)
    y = y_dev.transpose(0, 2, 1, 3).reshape(B, HP, HP)
    return np.ascontiguousarray(y, np.float32)



# revision 9
# speedup vs baseline: 1.4114x; 1.4114x over previous
"""Trainium2 Bass kernel for nn_DeconvDft2dLayer.

y = irfft2(gmf * rfft2(pad(x)))  with x (64,512,512), w (3,3), y (64,768,768).

Data-parallel over batch (8 samples per NeuronCore); per sample the 2-D FFTs
are DFT matmuls on the tensor engine (fp32r, full rate), restructured around
two symmetries the direct factorization misses:

1. gmf is even in the H-frequency: gmf[768-k, j] == gmf[k, j].  Writing
   P = S + conj(S'), M = S - conj(S') (S' the conjugate-mirror row transform)
   gives P = 2*(S1re @ C1), M = 2i*(S1im @ C1): the row DFT (stage B) needs
   only REAL x complex products (48 matmuls vs 96) and the inverse H-DFT
   (stage D) becomes cos/sin blocks against G-scaled data.
2. cos/sin mirror symmetry of the inverse transforms: U(768-n) = Ucos - Usin
   and y(., 768-m) = ycos + ysin, so stages D and E only compute 385-wide
   half-spectra and reconstruct mirrors with one vector add/sub each; the
   host undoes the resulting row/column permutation for free.

Per-sample: A 32, B 48, D 49, E 42 matmuls (171 vs baseline 344), all with
386/384-wide free dims (fp32r full rate).  Elementwise work (gmf scaling,
mirror combines) is spread over vector, gpsimd and scalar engines so the
tensor engine stays the only critical path.  Constants are host-built in
float64 and DMAed in SBUF tile layout; no cross-device communication.
"""
import os

import numpy as np

import concourse.bacc as bacc
import concourse.mybir as mybir
import concourse.tile as tile
from concourse.bass_utils import run_bass_kernel_spmd

F32 = mybir.dt.float32
F32R = mybir.dt.float32r

HP = 768          # padded grid
J = 385           # rfft half length (768//2+1)
JP = 386          # padded to even for fp32r free-dim constraint
NS = 8            # samples per core
NCORES = 8

LAST_EXEC_NS = None
LAST_RESULTS = None


def _build_constants(w):
    """Host-side constants (float64 -> float32), packed in SBUF tile layout."""
    w = np.asarray(w, np.float64)
    hm1 = np.zeros((HP, HP)); hm1[:3, :3] = w
    gm1f = 1.0 / np.fft.rfft2(hm1)
    gm2f = np.roll(gm1f[::-1, :], shift=1, axis=0)
    gm3f = np.roll(gm1f[:, ::-1], shift=1, axis=1)
    gm4f = np.roll(gm3f[::-1, :], shift=1, axis=0)
    gmf = (gm1f * gm2f) * (gm3f * gm4f)          # (768, 385) complex, even in k
    gre, gim = gmf.real, gmf.imag

    th = 2 * np.pi / HP
    h = np.arange(512)
    k = np.arange(J)
    phA = np.exp(-1j * th * np.outer(h + 128, k))   # (512, 385)
    ca = np.zeros((2, 512, JP))
    ca[0, :, :J] = phA.real
    ca[1, :, :J] = phA.imag

    jj = np.arange(J)
    phB = np.exp(-1j * th * np.outer(h + 128, jj))  # same table, w axis
    cb = np.zeros((2, 512, JP))
    cb[0, :, :J] = phB.real
    cb[1, :, :J] = phB.imag

    # chunk-row layout: rows 0..383 = P rows k 0..383 (from S1re),
    # rows 384..766 = M rows k 1..383 (from S1im), row 767 = P row k=384.
    kP = np.arange(384)
    kM = np.arange(1, 384)
    GA = np.zeros((HP, JP)); GB = np.zeros((HP, JP))
    s = np.where(kP == 0, 1.0, 2.0)[:, None]
    GA[:384, :J] = s * gre[kP]; GB[:384, :J] = -s * gim[kP]
    GA[384:767, :J] = -2 * gre[kM]; GB[384:767, :J] = 2 * gim[kM]
    GA[767, :J] = gre[384]; GB[767, :J] = -gim[384]
    # (GD == GA and GC == -GB, so two tables suffice.)

    npr = np.arange(J)
    ctcos = np.zeros((HP, JP)); ctsin = np.zeros((HP, JP))
    ctcos[:384, :J] = np.cos(th * np.outer(kP, npr)) / HP**2
    ctsin[384:767, :J] = np.sin(th * np.outer(kM, npr)) / HP**2
    ctcos[767, :J] = np.cos(np.pi * npr) / HP**2
    ctc = ctcos.reshape(6, 128, JP)[[0, 1, 2, 5]]   # chunks with cos rows
    cts = ctsin.reshape(6, 128, JP)[3:6]            # chunks with sin rows

    wj = np.where((jj == 0) | (jj == 384), 1.0, 2.0)[:, None]
    ec = np.zeros((384, JP)); es = np.zeros((384, JP))
    ec[:, :J] = wj[:384] * np.cos(th * np.outer(jj[:384], npr))
    es[:, :J] = wj[:384] * np.sin(th * np.outer(jj[:384], npr))
    ec384 = np.zeros((1, JP))
    ec384[0, :J] = np.cos(np.pi * npr)              # wj(384) = 1

    f = np.float32
    return {
        # packed to SBUF layouts: leading dim = partition
        "ca": np.ascontiguousarray(ca.reshape(2, 4, 128, JP).transpose(2, 0, 1, 3), f),
        "cb": np.ascontiguousarray(cb.reshape(2, 4, 128, JP).transpose(2, 0, 1, 3), f),
        "gt": np.ascontiguousarray(
            np.stack([GA, GB]).reshape(2, 6, 128, JP).transpose(2, 0, 1, 3), f),
        "ctc": np.ascontiguousarray(ctc.transpose(1, 0, 2), f),
        "cts": np.ascontiguousarray(cts.transpose(1, 0, 2), f),
        "ec": np.ascontiguousarray(ec.reshape(3, 128, JP).transpose(1, 0, 2), f),
        "es": np.ascontiguousarray(es.reshape(3, 128, JP).transpose(1, 0, 2), f),
        "ec384": np.ascontiguousarray(ec384, f),
    }


def _build_program(ns=NS):
    nc = bacc.Bacc("TRN2", target_bir_lowering=False, debug=False,
                   num_devices=NCORES)
    x_ext = nc.declare_dram_parameter("x", [ns, 128, 4, 512], F32R, isOutput=False)
    y_ext = nc.declare_dram_parameter("y", [ns, 128, 6, HP], F32, isOutput=True)
    ca_ext = nc.declare_dram_parameter("ca", [128, 2, 4, JP], F32R, isOutput=False)
    cb_ext = nc.declare_dram_parameter("cb", [128, 2, 4, JP], F32R, isOutput=False)
    gt_ext = nc.declare_dram_parameter("gt", [128, 2, 6, JP], F32, isOutput=False)
    ctc_ext = nc.declare_dram_parameter("ctc", [128, 4, JP], F32R, isOutput=False)
    cts_ext = nc.declare_dram_parameter("cts", [128, 3, JP], F32R, isOutput=False)
    ec_ext = nc.declare_dram_parameter("ec", [128, 3, JP], F32R, isOutput=False)
    es_ext = nc.declare_dram_parameter("es", [128, 3, JP], F32R, isOutput=False)
    ec384_ext = nc.declare_dram_parameter("ec384", [1, JP], F32R, isOutput=False)

    MUL = mybir.AluOpType.mult
    ADD = mybir.AluOpType.add
    SUB = mybir.AluOpType.subtract

    with tile.TileContext(nc) as tc:
        with tc.tile_pool(name="const", bufs=1) as cpool, \
             tc.tile_pool(name="data", bufs=1) as dpool, \
             tc.tile_pool(name="xin", bufs=2) as xpool, \
             tc.tile_pool(name="yout", bufs=2) as ypool, \
             tc.tile_pool(name="scr", bufs=2) as spool, \
             tc.tile_pool(name="psum", bufs=8, space="PSUM") as ppool:

            # sample-0 input first so stage A can start during const loads
            xts = []
            xt0 = xpool.tile([128, 4, 512], F32R, tag="x")
            nc.sync.dma_start(out=xt0[:], in_=x_ext[0])
            xts.append(xt0)

            ca_t = cpool.tile([128, 2, 4, JP], F32R, tag="ca")
            nc.sync.dma_start(out=ca_t[:], in_=ca_ext[:])
            cb_t = cpool.tile([128, 2, 4, JP], F32R, tag="cb")
            nc.sync.dma_start(out=cb_t[:], in_=cb_ext[:])
            gt_t = cpool.tile([128, 2, 6, JP], F32, tag="gt")
            nc.sync.dma_start(out=gt_t[:], in_=gt_ext[:])
            ctc_t = cpool.tile([128, 4, JP], F32R, tag="ctc")
            nc.sync.dma_start(out=ctc_t[:], in_=ctc_ext[:])
            cts_t = cpool.tile([128, 3, JP], F32R, tag="cts")
            nc.sync.dma_start(out=cts_t[:], in_=cts_ext[:])
            ec_t = cpool.tile([128, 3, JP], F32R, tag="ec")
            nc.sync.dma_start(out=ec_t[:], in_=ec_ext[:])
            es_t = cpool.tile([128, 3, JP], F32R, tag="es")
            nc.sync.dma_start(out=es_t[:], in_=es_ext[:])
            ec384_t = cpool.tile([1, JP], F32R, tag="ec384")
            nc.sync.dma_start(out=ec384_t[:], in_=ec384_ext[:])

            def mm(ps, lhsT, rhs, start, stop):
                nc.tensor.matmul(ps, lhsT=lhsT, rhs=rhs, start=start, stop=stop)

            for b in range(ns):
                xt = xts[b]
                if b + 1 < ns:   # prefetch next sample
                    nxt = xpool.tile([128, 4, 512], F32R, tag="x")
                    nc.sync.dma_start(out=nxt[:], in_=x_ext[b + 1])
                    xts.append(nxt)

                s1b = dpool.tile([128, 4, HP], F32R, tag="s1b")
                dt = dpool.tile([128, 2, 6, JP], F32R, tag="dt")
                ul = dpool.tile([128, 2, 3, 384], F32R, tag="ul")
                uh = dpool.tile([128, 2, 3, 384], F32R, tag="uh")
                u384l = dpool.tile([1, 384], F32R, tag="u384l")
                u384h = dpool.tile([1, 384], F32R, tag="u384h")

                # ---- stage A: S1 = DFT_H(x), packed [w, k-layout] ----
                for comp in range(2):
                    for wc in range(4):
                        ps = ppool.tile([128, JP], F32, tag="ps")
                        for hc in range(4):
                            mm(ps[:], xt[:, hc, wc * 128:(wc + 1) * 128],
                               ca_t[:, comp, hc, :], hc == 0, hc == 3)
                        if comp == 0:
                            nc.scalar.copy(s1b[:, wc, 0:384], ps[:, 0:384])
                            nc.scalar.copy(s1b[:, wc, 767:768], ps[:, 384:385])
                        else:
                            nc.scalar.copy(s1b[:, wc, 384:767], ps[:, 1:384])

                # ---- stages B + C, chunk order puts cos-chunks first ----
                # gpsimd cannot read PSUM: chunks 0-2 run on DVE straight from
                # PSUM; chunks 5,3,4 are copied to SBUF by the scalar engine
                # and scaled on gpsimd.
                for c in (0, 1, 2, 5, 3, 4):
                    ps_e0 = ppool.tile([128, JP], F32, tag="ps")
                    ps_e1 = ppool.tile([128, JP], F32, tag="ps")
                    for wc in range(4):
                        mm(ps_e0[:], s1b[:, wc, c * 128:(c + 1) * 128],
                           cb_t[:, 0, wc, :], wc == 0, wc == 3)
                    for wc in range(4):
                        mm(ps_e1[:], s1b[:, wc, c * 128:(c + 1) * 128],
                           cb_t[:, 1, wc, :], wc == 0, wc == 3)
                    # stage C: D0 = GA*E0 + GB*E1 ; D1 = GA*E1 - GB*E0
                    d0 = dt[:, 0, c, :]
                    d1 = dt[:, 1, c, :]
                    if c < 3:
                        eng, e0, e1 = nc.vector, ps_e0, ps_e1
                        t0 = spool.tile([128, JP], F32, tag="scrv0")
                        t1 = spool.tile([128, JP], F32, tag="scrv1")
                    else:
                        e0 = spool.tile([128, JP], F32, tag="scre0")
                        e1 = spool.tile([128, JP], F32, tag="scre1")
                        nc.scalar.copy(e0[:], ps_e0[:])
                        nc.scalar.copy(e1[:], ps_e1[:])
                        eng = nc.gpsimd
                        t0 = spool.tile([128, JP], F32, tag="scrg0")
                        t1 = spool.tile([128, JP], F32, tag="scrg1")
                    eng.tensor_tensor(out=t0[:], in0=gt_t[:, 0, c, :], in1=e0[:], op=MUL)
                    eng.tensor_tensor(out=d0, in0=gt_t[:, 1, c, :], in1=e1[:], op=MUL)
                    eng.tensor_tensor(out=d0, in0=d0, in1=t0[:], op=ADD)
                    eng.tensor_tensor(out=t1[:], in0=gt_t[:, 1, c, :], in1=e0[:], op=MUL)
                    eng.tensor_tensor(out=d1, in0=gt_t[:, 0, c, :], in1=e1[:], op=MUL)
                    eng.tensor_tensor(out=d1, in0=d1, in1=t1[:], op=SUB)

                # ---- stage D: half-spectrum inverse H-DFT ----
                for jc in range(3):
                    jsl = slice(jc * 128, jc * 128 + 128)
                    for comp in range(2):
                        ps_c = ppool.tile([128, JP], F32, tag="ps")
                        ps_s = ppool.tile([128, JP], F32, tag="ps")
                        for i, c in enumerate((0, 1, 2, 5)):
                            mm(ps_c[:], dt[:, comp, c, jsl], ctc_t[:, i, :],
                               i == 0, i == 3)
                        for i, c in enumerate((3, 4, 5)):
                            mm(ps_s[:], dt[:, comp, c, jsl], cts_t[:, i, :],
                               i == 0, i == 2)
                        ssb = spool.tile([128, JP], F32, tag="scrd")
                        nc.scalar.copy(ssb[:], ps_s[:])
                        nc.vector.tensor_tensor(out=ul[:, comp, jc, :],
                                                in0=ps_c[:, 0:384], in1=ssb[:, 0:384], op=ADD)
                        nc.vector.tensor_tensor(out=uh[:, comp, jc, :],
                                                in0=ps_c[:, 1:385], in1=ssb[:, 1:385], op=SUB)

                # j = 384 column (W-Nyquist): only the real row feeds stage E
                ps_c4 = ppool.tile([2, JP], F32, tag="ps")
                ps_s4 = ppool.tile([2, JP], F32, tag="ps")
                for i, c in enumerate((0, 1, 2, 5)):
                    mm(ps_c4[:], dt[:, :, c, 384], ctc_t[:, i, :], i == 0, i == 3)
                for i, c in enumerate((3, 4, 5)):
                    mm(ps_s4[:], dt[:, :, c, 384], cts_t[:, i, :], i == 0, i == 2)
                s4 = spool.tile([1, JP], F32, tag="scr4")
                nc.scalar.copy(s4[:], ps_s4[0:1, :])
                nc.vector.tensor_tensor(out=u384l[:], in0=ps_c4[0:1, 0:384],
                                        in1=s4[:, 0:384], op=ADD)
                nc.vector.tensor_tensor(out=u384h[:], in0=ps_c4[0:1, 1:385],
                                        in1=s4[:, 1:385], op=SUB)

                # ---- stage E: half-spectrum inverse W-DFT + mirror ----
                for hi, (ut, u384) in enumerate(((ul, u384l), (uh, u384h))):
                    for nch in range(3):
                        nsl = slice(nch * 128, nch * 128 + 128)
                        ps_yc = ppool.tile([128, JP], F32, tag="ps")
                        ps_ys = ppool.tile([128, JP], F32, tag="ps")
                        for jc in range(3):
                            mm(ps_yc[:], ut[:, 0, jc, nsl], ec_t[:, jc, :],
                               jc == 0, False)
                        mm(ps_yc[:], u384[:, nsl], ec384_t[:], False, True)
                        for jc in range(3):
                            mm(ps_ys[:], ut[:, 1, jc, nsl], es_t[:, jc, :],
                               jc == 0, jc == 2)
                        ytc = ypool.tile([128, HP], F32, tag="y")
                        ysb = spool.tile([128, JP], F32, tag="scry")
                        nc.scalar.copy(ysb[:], ps_ys[:])
                        nc.vector.tensor_tensor(out=ytc[:, 0:384], in0=ps_yc[:, 0:384],
                                                in1=ysb[:, 0:384], op=SUB)
                        nc.vector.tensor_tensor(out=ytc[:, 384:768], in0=ps_yc[:, 1:385],
                                                in1=ysb[:, 1:385], op=ADD)
                        nc.sync.dma_start(out=y_ext[b, :, hi * 3 + nch, :], in_=ytc[:])

    nc.compile()
    return nc


_PROGRAM_CACHE = {}


def kernel(x, w, trace=False):
    global LAST_EXEC_NS, LAST_RESULTS
    x = np.asarray(x, np.float32)
    B = x.shape[0]
    # pack to the SBUF tile layout: x_dev[b, p, c, w] = x[b, c*128+p, w]
    x_dev = np.ascontiguousarray(
        x.reshape(B, 4, 128, 512).transpose(0, 2, 1, 3))
    consts = _build_constants(w)
    if NS not in _PROGRAM_CACHE:
        _PROGRAM_CACHE[NS] = _build_program(NS)
    nc = _PROGRAM_CACHE[NS]
    in_maps = []
    for core in range(NCORES):
        m = {"x": x_dev[core * NS:(core + 1) * NS]}
        m.update(consts)
        in_maps.append(m)
    if trace:
        os.environ.pop("BASS_NEVER_TRACE", None)
        res = run_bass_kernel_spmd(nc, in_maps, list(range(NCORES)), trace=True)
    else:
        # profiling needs the antenv NTFF shim; never let a stray BASS_TRACE
        # env var route us down that path during plain runs
        os.environ["BASS_NEVER_TRACE"] = "1"
        try:
            res = run_bass_kernel_spmd(nc, in_maps, list(range(NCORES)), trace=False)
        finally:
            os.environ.pop("BASS_NEVER_TRACE", None)
    LAST_EXEC_NS = res.exec_time_ns
    LAST_RESULTS = res
    # unshard: y_dev[b, p, plane, t] -> y[b, n, m] undoing the mirror packing
    y_dev = np.concatenate([res.results[i]["y"] for i in range(NCORES)], axis=0)
    cols = np.concatenate([np.arange(384), 1151 - np.arange(384, 768)])
    inv = np.argsort(cols)          # y[..., m] = dev[..., inv[m]]
    y = np.empty((B, HP, HP), np.float32)
    for nch in range(3):
        y[:, nch * 128:(nch + 1) * 128, :] = y_dev[:, :, nch, inv]
        # high plane nch: row p holds n = 767 - nch*128 - p
        lo = 640 - nch * 128
        y[:, lo:lo + 128, :] = y_dev[:, ::-1, 3 + nch, inv]
    return np.ascontiguousarray(y)


# revision 11
# speedup vs baseline: 1.8338x; 1.2993x over previous
"""Trainium2 Bass kernel for nn_DeconvDft2dLayer.

y = irfft2(gmf * rfft2(pad(x)))  with x (64,512,512), w (3,3), y (64,768,768).

Data-parallel over batch (8 samples per NeuronCore); per sample the 2-D FFTs
are DFT matmuls on the tensor engine (bf16 operands, fp32 PSUM), restructured
around two symmetries the direct factorization misses:

1. gmf is even in the H-frequency: gmf[768-k, j] == gmf[k, j].  Writing
   P = S + conj(S'), M = S - conj(S') (S' the conjugate-mirror row transform)
   gives P = 2*(S1re @ C1), M = 2i*(S1im @ C1): the row DFT (stage B) needs
   only REAL x complex products (48 matmuls vs 96) and the inverse H-DFT
   (stage D) becomes cos/sin blocks against G-scaled data (stage C).
2. cos/sin mirror symmetry of the inverse transforms: U(768-n) = Ucos - Usin
   and y(., 768-m) = ycos + ysin, so stages D and E only compute 385-wide
   half-spectra and reconstruct mirrors with one vector add/sub each; the
   host undoes the resulting row/column permutation for free.

Per-sample: A 32, B 48, D 49, E 42 matmuls (171 vs 344 for the direct
factorization), all 384/386-wide.  The sample loop is software-pipelined one
deep (tensor order A(b), D(b-1), B(b), E(b-1)) so stage C's elementwise work
— which must funnel through the scalar engine (PSUM->SBUF bf16 copies, since
gpsimd cannot read PSUM and DVE reads it at full-width rate) and the
DVE/gpsimd engines — overlaps the previous sample's D/E matmuls instead of
stalling the tensor engine.  bf16 keeps LDWEIGHTS off the critical path and
doubles DVE throughput for the all-SBUF ops.  Constants are host-built in
float64 and DMAed in SBUF tile layout; no cross-device communication.
"""
import os

import ml_dtypes
import numpy as np

import concourse.bacc as bacc
import concourse.mybir as mybir
import concourse.tile as tile
from concourse.bass_utils import run_bass_kernel_spmd

F32 = mybir.dt.float32
BF16 = mybir.dt.bfloat16
NPBF16 = ml_dtypes.bfloat16

HP = 768          # padded grid
J = 385           # rfft half length (768//2+1)
JP = 386          # padded to even free dims
NS = 8            # samples per core
NCORES = 8

LAST_EXEC_NS = None
LAST_RESULTS = None


def _build_constants(w):
    """Host-side constants (float64 -> bf16), packed in SBUF tile layout."""
    w = np.asarray(w, np.float64)
    hm1 = np.zeros((HP, HP)); hm1[:3, :3] = w
    gm1f = 1.0 / np.fft.rfft2(hm1)
    gm2f = np.roll(gm1f[::-1, :], shift=1, axis=0)
    gm3f = np.roll(gm1f[:, ::-1], shift=1, axis=1)
    gm4f = np.roll(gm3f[::-1, :], shift=1, axis=0)
    gmf = (gm1f * gm2f) * (gm3f * gm4f)          # (768, 385) complex, even in k
    gre, gim = gmf.real, gmf.imag

    th = 2 * np.pi / HP
    h = np.arange(512)
    k = np.arange(J)
    phA = np.exp(-1j * th * np.outer(h + 128, k))   # (512, 385)
    ca = np.zeros((2, 512, JP))
    ca[0, :, :J] = phA.real
    ca[1, :, :J] = phA.imag
    cb = ca                                          # same table along w

    # chunk-row layout: rows 0..383 = P rows k 0..383 (from S1re),
    # rows 384..766 = M rows k 1..383 (from S1im), row 767 = P row k=384.
    kP = np.arange(384)
    kM = np.arange(1, 384)
    GA = np.zeros((HP, JP)); GB = np.zeros((HP, JP))
    s = np.where(kP == 0, 1.0, 2.0)[:, None]
    GA[:384, :J] = s * gre[kP]; GB[:384, :J] = -s * gim[kP]
    GA[384:767, :J] = -2 * gre[kM]; GB[384:767, :J] = 2 * gim[kM]
    GA[767, :J] = gre[384]; GB[767, :J] = -gim[384]
    # (GD == GA and GC == -GB, so two tables suffice.)

    npr = np.arange(J)
    ctcos = np.zeros((HP, JP)); ctsin = np.zeros((HP, JP))
    ctcos[:384, :J] = np.cos(th * np.outer(kP, npr)) / HP**2
    ctsin[384:767, :J] = np.sin(th * np.outer(kM, npr)) / HP**2
    ctcos[767, :J] = np.cos(np.pi * npr) / HP**2
    ctc = ctcos.reshape(6, 128, JP)[[0, 1, 2, 5]]   # chunks with cos rows
    cts = ctsin.reshape(6, 128, JP)[3:6]            # chunks with sin rows

    jj = np.arange(J)
    wj = np.where((jj == 0) | (jj == 384), 1.0, 2.0)[:, None]
    ec = np.zeros((384, JP)); es = np.zeros((384, JP))
    ec[:, :J] = wj[:384] * np.cos(th * np.outer(jj[:384], npr))
    es[:, :J] = wj[:384] * np.sin(th * np.outer(jj[:384], npr))
    ec384 = np.zeros((1, JP))
    ec384[0, :J] = np.cos(np.pi * npr)              # wj(384) = 1

    f = NPBF16
    return {
        # packed to SBUF layouts: leading dim = partition
        "ca": np.ascontiguousarray(ca.reshape(2, 4, 128, JP).transpose(2, 0, 1, 3)).astype(f),
        "gt": np.ascontiguousarray(
            np.stack([GA, GB]).reshape(2, 6, 128, JP).transpose(2, 0, 1, 3)).astype(f),
        "ctc": np.ascontiguousarray(ctc.transpose(1, 0, 2)).astype(f),
        "cts": np.ascontiguousarray(cts.transpose(1, 0, 2)).astype(f),
        "ec": np.ascontiguousarray(ec.reshape(3, 128, JP).transpose(1, 0, 2)).astype(f),
        "es": np.ascontiguousarray(es.reshape(3, 128, JP).transpose(1, 0, 2)).astype(f),
        "ec384": np.ascontiguousarray(ec384).astype(f),
    }


GPSIMD_CHUNKS = (3, 4)          # stage-C chunks scaled on the Pool engine
CHUNK_ORDER = (3, 4, 5, 0, 1, 2)  # gpsimd chunks first: longest window


def _build_program(ns=NS):
    nc = bacc.Bacc("TRN2", target_bir_lowering=False, debug=False,
                   num_devices=NCORES)
    x_ext = nc.declare_dram_parameter("x", [ns, 128, 4, 512], BF16, isOutput=False)
    y_ext = nc.declare_dram_parameter("y", [ns, 128, 6, HP], F32, isOutput=True)
    ca_ext = nc.declare_dram_parameter("ca", [128, 2, 4, JP], BF16, isOutput=False)
    gt_ext = nc.declare_dram_parameter("gt", [128, 2, 6, JP], BF16, isOutput=False)
    ctc_ext = nc.declare_dram_parameter("ctc", [128, 4, JP], BF16, isOutput=False)
    cts_ext = nc.declare_dram_parameter("cts", [128, 3, JP], BF16, isOutput=False)
    ec_ext = nc.declare_dram_parameter("ec", [128, 3, JP], BF16, isOutput=False)
    es_ext = nc.declare_dram_parameter("es", [128, 3, JP], BF16, isOutput=False)
    ec384_ext = nc.declare_dram_parameter("ec384", [1, JP], BF16, isOutput=False)

    MUL = mybir.AluOpType.mult
    ADD = mybir.AluOpType.add
    SUB = mybir.AluOpType.subtract

    with tile.TileContext(nc) as tc:
        with tc.tile_pool(name="const", bufs=1) as cpool, \
             tc.tile_pool(name="data", bufs=2) as dpool, \
             tc.tile_pool(name="xin", bufs=2) as xpool, \
             tc.tile_pool(name="yout", bufs=2) as ypool, \
             tc.tile_pool(name="scr", bufs=2) as spool, \
             tc.tile_pool(name="psum", bufs=8, space="PSUM") as ppool:

            # sample-0 input first so stage A can start during const loads
            xts = []
            xt0 = xpool.tile([128, 4, 512], BF16, tag="x")
            nc.sync.dma_start(out=xt0[:], in_=x_ext[0])
            xts.append(xt0)

            ca_t = cpool.tile([128, 2, 4, JP], BF16, tag="ca")
            nc.sync.dma_start(out=ca_t[:], in_=ca_ext[:])
            gt_t = cpool.tile([128, 2, 6, JP], BF16, tag="gt")
            nc.sync.dma_start(out=gt_t[:], in_=gt_ext[:])
            ctc_t = cpool.tile([128, 4, JP], BF16, tag="ctc")
            nc.sync.dma_start(out=ctc_t[:], in_=ctc_ext[:])
            cts_t = cpool.tile([128, 3, JP], BF16, tag="cts")
            nc.sync.dma_start(out=cts_t[:], in_=cts_ext[:])
            ec_t = cpool.tile([128, 3, JP], BF16, tag="ec")
            nc.sync.dma_start(out=ec_t[:], in_=ec_ext[:])
            es_t = cpool.tile([128, 3, JP], BF16, tag="es")
            nc.sync.dma_start(out=es_t[:], in_=es_ext[:])
            ec384_t = cpool.tile([1, JP], BF16, tag="ec384")
            nc.sync.dma_start(out=ec384_t[:], in_=ec384_ext[:])

            def mm(ps, lhsT, rhs, start, stop):
                nc.tensor.matmul(ps, lhsT=lhsT, rhs=rhs, start=start, stop=stop)

            def emit_A(b, xt):
                """Stage A: S1 = DFT_H(x), packed [w, k-layout]."""
                s1b = dpool.tile([128, 4, HP], BF16, tag="s1b")
                for comp in range(2):
                    for wc in range(4):
                        ps = ppool.tile([128, JP], F32, tag="ps")
                        for hc in range(4):
                            mm(ps[:], xt[:, hc, wc * 128:(wc + 1) * 128],
                               ca_t[:, comp, hc, :], hc == 0, hc == 3)
                        if comp == 0:
                            nc.scalar.copy(s1b[:, wc, 0:384], ps[:, 0:384])
                            nc.scalar.copy(s1b[:, wc, 767:768], ps[:, 384:385])
                        else:
                            nc.scalar.copy(s1b[:, wc, 384:767], ps[:, 1:384])
                return s1b

            def emit_BC(b, s1b):
                """Stage B matmuls + stage C scaling.
                All chunk PSUMs go through scalar-engine bf16 copies so the
                scale/accumulate ops run all-SBUF-bf16 (2x DVE mode, and
                gpsimd legality)."""
                dt = dpool.tile([128, 2, 6, JP], BF16, tag="dt")
                for c in CHUNK_ORDER:
                    ps_e0 = ppool.tile([128, JP], F32, tag="ps")
                    ps_e1 = ppool.tile([128, JP], F32, tag="ps")
                    for wc in range(4):
                        mm(ps_e0[:], s1b[:, wc, c * 128:(c + 1) * 128],
                           ca_t[:, 0, wc, :], wc == 0, wc == 3)
                    for wc in range(4):
                        mm(ps_e1[:], s1b[:, wc, c * 128:(c + 1) * 128],
                           ca_t[:, 1, wc, :], wc == 0, wc == 3)
                    # stage C: D0 = GA*E0 + GB*E1 ; D1 = GA*E1 - GB*E0
                    gp = c in GPSIMD_CHUNKS
                    eng = nc.gpsimd if gp else nc.vector
                    pre = "g" if gp else "v"
                    e0 = spool.tile([128, JP], BF16, tag=pre + "e0")
                    e1 = spool.tile([128, JP], BF16, tag=pre + "e1")
                    nc.scalar.copy(e0[:], ps_e0[:])
                    nc.scalar.copy(e1[:], ps_e1[:])
                    d0 = dt[:, 0, c, :]
                    d1 = dt[:, 1, c, :]
                    t0 = spool.tile([128, JP], BF16, tag=pre + "t0")
                    t1 = spool.tile([128, JP], BF16, tag=pre + "t1")
                    eng.tensor_tensor(out=t0[:], in0=gt_t[:, 0, c, :], in1=e0[:], op=MUL)
                    eng.tensor_tensor(out=d0, in0=gt_t[:, 1, c, :], in1=e1[:], op=MUL)
                    eng.tensor_tensor(out=d0, in0=d0, in1=t0[:], op=ADD)
                    eng.tensor_tensor(out=t1[:], in0=gt_t[:, 1, c, :], in1=e0[:], op=MUL)
                    eng.tensor_tensor(out=d1, in0=gt_t[:, 0, c, :], in1=e1[:], op=MUL)
                    eng.tensor_tensor(out=d1, in0=d1, in1=t1[:], op=SUB)
                return dt

            def emit_D(b, dt):
                """Stage D: half-spectrum inverse H-DFT + mirror combine.
                comp-major order so stage E's Ure inputs finish first."""
                ul = dpool.tile([128, 2, 3, 384], BF16, tag="ul")
                uh = dpool.tile([128, 2, 3, 384], BF16, tag="uh")
                u384l = dpool.tile([1, 384], BF16, tag="u384l")
                u384h = dpool.tile([1, 384], BF16, tag="u384h")
                for comp in range(2):
                    for jc in range(3):
                        jsl = slice(jc * 128, jc * 128 + 128)
                        ps_c = ppool.tile([128, JP], F32, tag="ps")
                        ps_s = ppool.tile([128, JP], F32, tag="ps")
                        for i, c in enumerate((0, 1, 2, 5)):
                            mm(ps_c[:], dt[:, comp, c, jsl], ctc_t[:, i, :],
                               i == 0, i == 3)
                        for i, c in enumerate((3, 4, 5)):
                            mm(ps_s[:], dt[:, comp, c, jsl], cts_t[:, i, :],
                               i == 0, i == 2)
                        ssb = spool.tile([128, JP], BF16, tag="scrd")
                        nc.scalar.copy(ssb[:], ps_s[:])
                        nc.vector.tensor_tensor(out=ul[:, comp, jc, :],
                                                in0=ps_c[:, 0:384], in1=ssb[:, 0:384], op=ADD)
                        nc.vector.tensor_tensor(out=uh[:, comp, jc, :],
                                                in0=ps_c[:, 1:385], in1=ssb[:, 1:385], op=SUB)

                # j = 384 column (W-Nyquist): only the real row feeds stage E
                ps_c4 = ppool.tile([2, JP], F32, tag="ps")
                ps_s4 = ppool.tile([2, JP], F32, tag="ps")
                for i, c in enumerate((0, 1, 2, 5)):
                    mm(ps_c4[:], dt[:, :, c, 384], ctc_t[:, i, :], i == 0, i == 3)
                for i, c in enumerate((3, 4, 5)):
                    mm(ps_s4[:], dt[:, :, c, 384], cts_t[:, i, :], i == 0, i == 2)
                s4 = spool.tile([1, JP], BF16, tag="scr4")
                nc.scalar.copy(s4[:], ps_s4[0:1, :])
                nc.vector.tensor_tensor(out=u384l[:], in0=ps_c4[0:1, 0:384],
                                        in1=s4[:, 0:384], op=ADD)
                nc.vector.tensor_tensor(out=u384h[:], in0=ps_c4[0:1, 1:385],
                                        in1=s4[:, 1:385], op=SUB)
                return ul, uh, u384l, u384h

            def emit_E(b, us):
                """Stage E: half-spectrum inverse W-DFT + mirror."""
                ul, uh, u384l, u384h = us
                for hi, (ut, u384) in enumerate(((ul, u384l), (uh, u384h))):
                    for nch in range(3):
                        nsl = slice(nch * 128, nch * 128 + 128)
                        ps_yc = ppool.tile([128, JP], F32, tag="ps")
                        ps_ys = ppool.tile([128, JP], F32, tag="ps")
                        for jc in range(3):
                            mm(ps_yc[:], ut[:, 0, jc, nsl], ec_t[:, jc, :],
                               jc == 0, False)
                        mm(ps_yc[:], u384[:, nsl], ec384_t[:], False, True)
                        for jc in range(3):
                            mm(ps_ys[:], ut[:, 1, jc, nsl], es_t[:, jc, :],
                               jc == 0, jc == 2)
                        ytc = ypool.tile([128, HP], F32, tag="y")
                        ysb = spool.tile([128, JP], BF16, tag="scry")
                        nc.scalar.copy(ysb[:], ps_ys[:])
                        nc.vector.tensor_tensor(out=ytc[:, 0:384], in0=ps_yc[:, 0:384],
                                                in1=ysb[:, 0:384], op=SUB)
                        nc.vector.tensor_tensor(out=ytc[:, 384:768], in0=ps_yc[:, 1:385],
                                                in1=ysb[:, 1:385], op=ADD)
                        nc.sync.dma_start(out=y_ext[b, :, hi * 3 + nch, :], in_=ytc[:])

            # one-sample software pipeline: tensor order A(b) D(b-1) B(b) E(b-1)
            dts = [None] * ns
            uss = [None] * ns
            for b in range(ns + 1):
                s1b = None
                if b < ns:
                    if b + 1 < ns:   # prefetch next sample
                        nxt = xpool.tile([128, 4, 512], BF16, tag="x")
                        nc.sync.dma_start(out=nxt[:], in_=x_ext[b + 1])
                        xts.append(nxt)
                    s1b = emit_A(b, xts[b])
                if b >= 1:
                    uss[b - 1] = emit_D(b - 1, dts[b - 1])
                if b < ns:
                    dts[b] = emit_BC(b, s1b)
                if b >= 1:
                    emit_E(b - 1, uss[b - 1])

    nc.compile()
    return nc


_PROGRAM_CACHE = {}


def kernel(x, w, trace=False):
    global LAST_EXEC_NS, LAST_RESULTS
    x = np.asarray(x, np.float32)
    B = x.shape[0]
    # pack to the SBUF tile layout: x_dev[b, p, c, w] = x[b, c*128+p, w]
    x_dev = np.ascontiguousarray(
        x.reshape(B, 4, 128, 512).transpose(0, 2, 1, 3)).astype(NPBF16)
    consts = _build_constants(w)
    if NS not in _PROGRAM_CACHE:
        _PROGRAM_CACHE[NS] = _build_program(NS)
    nc = _PROGRAM_CACHE[NS]
    in_maps = []
    for core in range(NCORES):
        m = {"x": x_dev[core * NS:(core + 1) * NS]}
        m.update(consts)
        in_maps.append(m)
    if trace:
        os.environ.pop("BASS_NEVER_TRACE", None)
        res = run_bass_kernel_spmd(nc, in_maps, list(range(NCORES)), trace=True)
    else:
        # profiling needs the antenv NTFF shim; never let a stray BASS_TRACE
        # env var route us down that path during plain runs
        os.environ["BASS_NEVER_TRACE"] = "1"
        try:
            res = run_bass_kernel_spmd(nc, in_maps, list(range(NCORES)), trace=False)
        finally:
            os.environ.pop("BASS_NEVER_TRACE", None)
    LAST_EXEC_NS = res.exec_time_ns
    LAST_RESULTS = res
    # unshard: y_dev[b, p, plane, t] -> y[b, n, m] undoing the mirror packing
    y_dev = np.concatenate([res.results[i]["y"] for i in range(NCORES)], axis=0)
    cols = np.concatenate([np.arange(384), 1151 - np.arange(384, 768)])
    inv = np.argsort(cols)          # y[..., m] = dev[..., inv[m]]
    y = np.empty((B, HP, HP), np.float32)
    for nch in range(3):
        y[:, nch * 128:(nch + 1) * 128, :] = y_dev[:, :, nch, inv]
        # high plane nch: row p holds n = 767 - nch*128 - p
        lo = 640 - nch * 128
        y[:, lo:lo + 128, :] = y_dev[:, ::-1, 3 + nch, inv]
    return np.ascontiguousarray(y)


# revision 14
# speedup vs baseline: 1.9900x; 1.0851x over previous
"""Trainium2 Bass kernel for nn_DeconvDft2dLayer.

y = irfft2(gmf * rfft2(pad(x)))  with x (64,512,512), w (3,3), y (64,768,768).

Data-parallel over batch (8 samples per NeuronCore); per sample the 2-D FFTs
are DFT matmuls on the tensor engine (bf16 operands, fp32 PSUM), restructured
around two symmetries the direct factorization misses:

1. gmf is even in the H-frequency: gmf[768-k, j] == gmf[k, j].  Writing
   P = S + conj(S'), M = S - conj(S') (S' the conjugate-mirror row transform)
   gives P = 2*(S1re @ C1), M = 2i*(S1im @ C1): the row DFT (stage B) needs
   only REAL x complex products (48 matmuls vs 96) and the inverse H-DFT
   (stage D) becomes cos/sin blocks against G-scaled data (stage C).
2. cos/sin mirror symmetry of the inverse transforms: U(768-n) = Ucos - Usin
   and y(., 768-m) = ycos + ysin, so stages D and E only compute 385-wide
   half-spectra and reconstruct mirrors with one vector add/sub each; the
   host undoes the resulting row/column permutation for free.

Per-sample: A 32, B 48, D 49, E 42 matmuls (171 vs 344 for the direct
factorization), all 384/386-wide.  The sample loop is software-pipelined one
deep (tensor order A(b), D(b-1), B(b), E(b-1)) so stage C's elementwise work
— which must funnel through the scalar engine (PSUM->SBUF bf16 copies, since
gpsimd cannot read PSUM and DVE reads it at full-width rate) and the
DVE/gpsimd engines — overlaps the previous sample's D/E matmuls instead of
stalling the tensor engine.  bf16 keeps LDWEIGHTS off the critical path and
doubles DVE throughput for the all-SBUF ops.  Constants are host-built in
float64 and DMAed in SBUF tile layout; no cross-device communication.
"""
import os

import ml_dtypes
import numpy as np

import concourse.bacc as bacc
import concourse.mybir as mybir
import concourse.tile as tile
from concourse.bass_utils import run_bass_kernel_spmd

F32 = mybir.dt.float32
BF16 = mybir.dt.bfloat16
NPBF16 = ml_dtypes.bfloat16

HP = 768          # padded grid
J = 385           # rfft half length (768//2+1)
JP = 386          # padded to even free dims
NS = 8            # samples per core
NCORES = 8

LAST_EXEC_NS = None
LAST_RESULTS = None


def _build_constants(w):
    """Host-side constants (float64 -> bf16), packed in SBUF tile layout."""
    w = np.asarray(w, np.float64)
    hm1 = np.zeros((HP, HP)); hm1[:3, :3] = w
    gm1f = 1.0 / np.fft.rfft2(hm1)
    gm2f = np.roll(gm1f[::-1, :], shift=1, axis=0)
    gm3f = np.roll(gm1f[:, ::-1], shift=1, axis=1)
    gm4f = np.roll(gm3f[::-1, :], shift=1, axis=0)
    gmf = (gm1f * gm2f) * (gm3f * gm4f)          # (768, 385) complex, even in k
    gre, gim = gmf.real, gmf.imag

    th = 2 * np.pi / HP
    h = np.arange(512)
    k = np.arange(J)
    phA = np.exp(-1j * th * np.outer(h + 128, k))   # (512, 385)
    ca = np.zeros((2, 512, JP))
    ca[0, :, :J] = phA.real
    ca[1, :, :J] = phA.imag
    cb = ca                                          # same table along w

    # chunk-row layout: rows 0..383 = P rows k 0..383 (from S1re),
    # rows 384..766 = M rows k 1..383 (from S1im), row 767 = P row k=384.
    kP = np.arange(384)
    kM = np.arange(1, 384)
    GA = np.zeros((HP, JP)); GB = np.zeros((HP, JP))
    s = np.where(kP == 0, 1.0, 2.0)[:, None]
    GA[:384, :J] = s * gre[kP]; GB[:384, :J] = -s * gim[kP]
    GA[384:767, :J] = -2 * gre[kM]; GB[384:767, :J] = 2 * gim[kM]
    GA[767, :J] = gre[384]; GB[767, :J] = -gim[384]
    # (GD == GA and GC == -GB, so two tables suffice.)

    npr = np.arange(J)
    ctcos = np.zeros((HP, JP)); ctsin = np.zeros((HP, JP))
    ctcos[:384, :J] = np.cos(th * np.outer(kP, npr)) / HP**2
    ctsin[384:767, :J] = np.sin(th * np.outer(kM, npr)) / HP**2
    ctcos[767, :J] = np.cos(np.pi * npr) / HP**2
    ctc = ctcos.reshape(6, 128, JP)[[0, 1, 2, 5]]   # chunks with cos rows
    cts = ctsin.reshape(6, 128, JP)[3:6]            # chunks with sin rows

    jj = np.arange(J)
    wj = np.where((jj == 0) | (jj == 384), 1.0, 2.0)[:, None]
    ec = np.zeros((384, JP)); es = np.zeros((384, JP))
    ec[:, :J] = wj[:384] * np.cos(th * np.outer(jj[:384], npr))
    es[:, :J] = wj[:384] * np.sin(th * np.outer(jj[:384], npr))
    ec384 = np.zeros((1, JP))
    ec384[0, :J] = np.cos(np.pi * npr)              # wj(384) = 1

    f = NPBF16
    return {
        # packed to SBUF layouts: leading dim = partition
        "ca": np.ascontiguousarray(ca.reshape(2, 4, 128, JP).transpose(2, 0, 1, 3)).astype(f),
        "gt": np.ascontiguousarray(
            np.stack([GA, GB]).reshape(2, 6, 128, JP).transpose(2, 0, 1, 3)).astype(f),
        "ctc": np.ascontiguousarray(ctc.transpose(1, 0, 2)).astype(f),
        "cts": np.ascontiguousarray(cts.transpose(1, 0, 2)).astype(f),
        "ec": np.ascontiguousarray(ec.reshape(3, 128, JP).transpose(1, 0, 2)).astype(f),
        "es": np.ascontiguousarray(es.reshape(3, 128, JP).transpose(1, 0, 2)).astype(f),
        "ec384": np.ascontiguousarray(ec384).astype(f),
    }


GPSIMD_CHUNKS = (5, 3, 4)       # stage-C chunks scaled on the Pool engine
# DVE chunks (re-part, ready after comp0 A-copies) first so stage B never
# waits on the scalar engine; gpsimd chunks trail with a full-iteration window
CHUNK_ORDER = (0, 1, 2, 5, 3, 4)


def _build_program(ns=NS):
    nc = bacc.Bacc("TRN2", target_bir_lowering=False, debug=False,
                   num_devices=NCORES)
    x_ext = nc.declare_dram_parameter("x", [ns, 128, 4, 512], BF16, isOutput=False)
    y_ext = nc.declare_dram_parameter("y", [ns, 128, 6, HP], F32, isOutput=True)
    ca_ext = nc.declare_dram_parameter("ca", [128, 2, 4, JP], BF16, isOutput=False)
    gt_ext = nc.declare_dram_parameter("gt", [128, 2, 6, JP], BF16, isOutput=False)
    ctc_ext = nc.declare_dram_parameter("ctc", [128, 4, JP], BF16, isOutput=False)
    cts_ext = nc.declare_dram_parameter("cts", [128, 3, JP], BF16, isOutput=False)
    ec_ext = nc.declare_dram_parameter("ec", [128, 3, JP], BF16, isOutput=False)
    es_ext = nc.declare_dram_parameter("es", [128, 3, JP], BF16, isOutput=False)
    ec384_ext = nc.declare_dram_parameter("ec384", [1, JP], BF16, isOutput=False)

    MUL = mybir.AluOpType.mult
    ADD = mybir.AluOpType.add
    SUB = mybir.AluOpType.subtract

    with tile.TileContext(nc) as tc:
        with tc.tile_pool(name="const", bufs=1) as cpool, \
             tc.tile_pool(name="data", bufs=2) as dpool, \
             tc.tile_pool(name="xin", bufs=2) as xpool, \
             tc.tile_pool(name="yout", bufs=2) as ypool, \
             tc.tile_pool(name="scr", bufs=2) as spool, \
             tc.tile_pool(name="psum", bufs=8, space="PSUM") as ppool:

            # sample-0 input first so stage A can start during const loads
            xts = []
            xt0 = xpool.tile([128, 4, 512], BF16, tag="x")
            nc.sync.dma_start(out=xt0[:], in_=x_ext[0])
            xts.append(xt0)

            ca_t = cpool.tile([128, 2, 4, JP], BF16, tag="ca")
            nc.sync.dma_start(out=ca_t[:], in_=ca_ext[:])
            gt_t = cpool.tile([128, 2, 6, JP], BF16, tag="gt")
            nc.sync.dma_start(out=gt_t[:], in_=gt_ext[:])
            ctc_t = cpool.tile([128, 4, JP], BF16, tag="ctc")
            nc.sync.dma_start(out=ctc_t[:], in_=ctc_ext[:])
            cts_t = cpool.tile([128, 3, JP], BF16, tag="cts")
            nc.sync.dma_start(out=cts_t[:], in_=cts_ext[:])
            ec_t = cpool.tile([128, 3, JP], BF16, tag="ec")
            nc.sync.dma_start(out=ec_t[:], in_=ec_ext[:])
            es_t = cpool.tile([128, 3, JP], BF16, tag="es")
            nc.sync.dma_start(out=es_t[:], in_=es_ext[:])
            ec384_t = cpool.tile([1, JP], BF16, tag="ec384")
            nc.sync.dma_start(out=ec384_t[:], in_=ec384_ext[:])

            def mm(ps, lhsT, rhs, start, stop):
                nc.tensor.matmul(ps, lhsT=lhsT, rhs=rhs, start=start, stop=stop)

            def emit_A(b, xt):
                """Stage A: S1 = DFT_H(x), packed [w, k-layout]."""
                s1b = dpool.tile([128, 4, HP], BF16, tag="s1b")
                for comp in range(2):
                    for wc in range(4):
                        ps = ppool.tile([128, JP], F32, tag="ps")
                        for hc in range(4):
                            mm(ps[:], xt[:, hc, wc * 128:(wc + 1) * 128],
                               ca_t[:, comp, hc, :], hc == 0, hc == 3)
                        if comp == 0:
                            nc.scalar.copy(s1b[:, wc, 0:384], ps[:, 0:384])
                            nc.scalar.copy(s1b[:, wc, 767:768], ps[:, 384:385])
                        else:
                            nc.scalar.copy(s1b[:, wc, 384:767], ps[:, 1:384])
                return s1b

            def emit_BC(b, s1b):
                """Stage B matmuls + stage C scaling.
                All chunk PSUMs go through scalar-engine bf16 copies so the
                scale/accumulate ops run all-SBUF-bf16 (gpsimd legality, 16-bit
                DVE mode).  The DVE chunks' ops are returned as a deferred
                closure: they are emitted after emit_E so the previous
                sample's E+- combines sit ahead of them in the DVE queue and
                release their PSUM banks before the next sample's stage A
                needs them."""
                dt = dpool.tile([128, 2, 6, JP], BF16, tag="dt")
                deferred = []
                for c in CHUNK_ORDER:
                    ps_e0 = ppool.tile([128, JP], F32, tag="ps")
                    ps_e1 = ppool.tile([128, JP], F32, tag="ps")
                    for wc in range(4):
                        mm(ps_e0[:], s1b[:, wc, c * 128:(c + 1) * 128],
                           ca_t[:, 0, wc, :], wc == 0, wc == 3)
                    for wc in range(4):
                        mm(ps_e1[:], s1b[:, wc, c * 128:(c + 1) * 128],
                           ca_t[:, 1, wc, :], wc == 0, wc == 3)
                    # stage C: D0 = GA*E0 + GB*E1 ; D1 = GA*E1 - GB*E0
                    gp = c in GPSIMD_CHUNKS
                    eng = nc.gpsimd if gp else nc.vector
                    pre = "g" if gp else "v"
                    e0 = spool.tile([128, JP], BF16, tag=pre + "e0")
                    e1 = spool.tile([128, JP], BF16, tag=pre + "e1")
                    nc.scalar.copy(e0[:], ps_e0[:])
                    nc.scalar.copy(e1[:], ps_e1[:])

                    def scale(c=c, eng=eng, pre=pre, e0=e0, e1=e1):
                        d0 = dt[:, 0, c, :]
                        d1 = dt[:, 1, c, :]
                        t0 = spool.tile([128, JP], BF16, tag=pre + "t0")
                        t1 = spool.tile([128, JP], BF16, tag=pre + "t1")
                        eng.tensor_tensor(out=t0[:], in0=gt_t[:, 0, c, :], in1=e0[:], op=MUL)
                        eng.tensor_tensor(out=d0, in0=gt_t[:, 1, c, :], in1=e1[:], op=MUL)
                        eng.tensor_tensor(out=d0, in0=d0, in1=t0[:], op=ADD)
                        eng.tensor_tensor(out=t1[:], in0=gt_t[:, 1, c, :], in1=e0[:], op=MUL)
                        eng.tensor_tensor(out=d1, in0=gt_t[:, 0, c, :], in1=e1[:], op=MUL)
                        eng.tensor_tensor(out=d1, in0=d1, in1=t1[:], op=SUB)

                    if gp:
                        scale()          # Pool queue is independent: emit now
                    else:
                        deferred.append(scale)
                return dt, deferred

            def emit_D(b, dt):
                """Stage D: half-spectrum inverse H-DFT + mirror combine.
                comp-major order so stage E's Ure inputs finish first."""
                ul = dpool.tile([128, 2, 3, 384], BF16, tag="ul")
                uh = dpool.tile([128, 2, 3, 384], BF16, tag="uh")
                u384l = dpool.tile([1, 384], BF16, tag="u384l")
                u384h = dpool.tile([1, 384], BF16, tag="u384h")
                for comp in range(2):
                    for jc in range(3):
                        jsl = slice(jc * 128, jc * 128 + 128)
                        ps_c = ppool.tile([128, JP], F32, tag="ps")
                        ps_s = ppool.tile([128, JP], F32, tag="ps")
                        for i, c in enumerate((0, 1, 2, 5)):
                            mm(ps_c[:], dt[:, comp, c, jsl], ctc_t[:, i, :],
                               i == 0, i == 3)
                        for i, c in enumerate((3, 4, 5)):
                            mm(ps_s[:], dt[:, comp, c, jsl], cts_t[:, i, :],
                               i == 0, i == 2)
                        ssb = spool.tile([128, JP], BF16, tag="scrd")
                        nc.scalar.copy(ssb[:], ps_s[:])
                        nc.vector.tensor_tensor(out=ul[:, comp, jc, :],
                                                in0=ps_c[:, 0:384], in1=ssb[:, 0:384], op=ADD)
                        nc.vector.tensor_tensor(out=uh[:, comp, jc, :],
                                                in0=ps_c[:, 1:385], in1=ssb[:, 1:385], op=SUB)

                # j = 384 column (W-Nyquist): only the real row feeds stage E
                ps_c4 = ppool.tile([2, JP], F32, tag="ps")
                ps_s4 = ppool.tile([2, JP], F32, tag="ps")
                for i, c in enumerate((0, 1, 2, 5)):
                    mm(ps_c4[:], dt[:, :, c, 384], ctc_t[:, i, :], i == 0, i == 3)
                for i, c in enumerate((3, 4, 5)):
                    mm(ps_s4[:], dt[:, :, c, 384], cts_t[:, i, :], i == 0, i == 2)
                s4 = spool.tile([1, JP], BF16, tag="scr4")
                nc.scalar.copy(s4[:], ps_s4[0:1, :])
                nc.vector.tensor_tensor(out=u384l[:], in0=ps_c4[0:1, 0:384],
                                        in1=s4[:, 0:384], op=ADD)
                nc.vector.tensor_tensor(out=u384h[:], in0=ps_c4[0:1, 1:385],
                                        in1=s4[:, 1:385], op=SUB)
                return ul, uh, u384l, u384h

            def emit_E(b, us):
                """Stage E: half-spectrum inverse W-DFT + mirror."""
                ul, uh, u384l, u384h = us
                for hi, (ut, u384) in enumerate(((ul, u384l), (uh, u384h))):
                    for nch in range(3):
                        nsl = slice(nch * 128, nch * 128 + 128)
                        ps_yc = ppool.tile([128, JP], F32, tag="ps")
                        ps_ys = ppool.tile([128, JP], F32, tag="ps")
                        for jc in range(3):
                            mm(ps_yc[:], ut[:, 0, jc, nsl], ec_t[:, jc, :],
                               jc == 0, False)
                        mm(ps_yc[:], u384[:, nsl], ec384_t[:], False, True)
                        for jc in range(3):
                            mm(ps_ys[:], ut[:, 1, jc, nsl], es_t[:, jc, :],
                               jc == 0, jc == 2)
                        ytc = ypool.tile([128, HP], F32, tag="y")
                        ysb = spool.tile([128, JP], BF16, tag="scry")
                        nc.scalar.copy(ysb[:], ps_ys[:])
                        nc.vector.tensor_tensor(out=ytc[:, 0:384], in0=ps_yc[:, 0:384],
                                                in1=ysb[:, 0:384], op=SUB)
                        nc.vector.tensor_tensor(out=ytc[:, 384:768], in0=ps_yc[:, 1:385],
                                                in1=ysb[:, 1:385], op=ADD)
                        nc.sync.dma_start(out=y_ext[b, :, hi * 3 + nch, :], in_=ytc[:])

            # one-sample software pipeline: tensor order A(b) D(b-1) B(b) E(b-1)
            dts = [None] * ns
            uss = [None] * ns
            for b in range(ns + 1):
                s1b = None
                if b < ns:
                    if b + 1 < ns:   # prefetch next sample
                        nxt = xpool.tile([128, 4, 512], BF16, tag="x")
                        nc.sync.dma_start(out=nxt[:], in_=x_ext[b + 1])
                        xts.append(nxt)
                    s1b = emit_A(b, xts[b])
                if b >= 1:
                    uss[b - 1] = emit_D(b - 1, dts[b - 1])
                deferred = []
                if b < ns:
                    dts[b], deferred = emit_BC(b, s1b)
                if b >= 1:
                    emit_E(b - 1, uss[b - 1])
                for fn in deferred:
                    fn()

    nc.compile()
    return nc


_PROGRAM_CACHE = {}


def kernel(x, w, trace=False):
    global LAST_EXEC_NS, LAST_RESULTS
    x = np.asarray(x, np.float32)
    B = x.shape[0]
    # pack to the SBUF tile layout: x_dev[b, p, c, w] = x[b, c*128+p, w]
    x_dev = np.ascontiguousarray(
        x.reshape(B, 4, 128, 512).transpose(0, 2, 1, 3)).astype(NPBF16)
    consts = _build_constants(w)
    if NS not in _PROGRAM_CACHE:
        _PROGRAM_CACHE[NS] = _build_program(NS)
    nc = _PROGRAM_CACHE[NS]
    in_maps = []
    for core in range(NCORES):
        m = {"x": x_dev[core * NS:(core + 1) * NS]}
        m.update(consts)
        in_maps.append(m)
    if trace:
        os.environ.pop("BASS_NEVER_TRACE", None)
        res = run_bass_kernel_spmd(nc, in_maps, list(range(NCORES)), trace=True)
    else:
        # profiling needs the antenv NTFF shim; never let a stray BASS_TRACE
        # env var route us down that path during plain runs
        os.environ["BASS_NEVER_TRACE"] = "1"
        try:
            res = run_bass_kernel_spmd(nc, in_maps, list(range(NCORES)), trace=False)
        finally:
            os.environ.pop("BASS_NEVER_TRACE", None)
    LAST_EXEC_NS = res.exec_time_ns
    LAST_RESULTS = res
    # unshard: y_dev[b, p, plane, t] -> y[b, n, m] undoing the mirror packing
    y_dev = np.concatenate([res.results[i]["y"] for i in range(NCORES)], axis=0)
    cols = np.concatenate([np.arange(384), 1151 - np.arange(384, 768)])
    inv = np.argsort(cols)          # y[..., m] = dev[..., inv[m]]
    y = np.empty((B, HP, HP), np.float32)
    for nch in range(3):
        y[:, nch * 128:(nch + 1) * 128, :] = y_dev[:, :, nch, inv]
        # high plane nch: row p holds n = 767 - nch*128 - p
        lo = 640 - nch * 128
        y[:, lo:lo + 128, :] = y_dev[:, ::-1, 3 + nch, inv]
    return np.ascontiguousarray(y)
